# revision 38
# baseline (speedup 1.0000x reference)
"""Multi-head self-attention (B=4, S=2048, D=1024, H=8) on 8 TRN2 NeuronCores.

Sharding: core c -> batch b=c//2, head-group g=c%2 (4 heads/core).
Each core computes its 4 heads' attention output [512, 2048] (transposed,
head-major); the host gathers/reassembles the full [B, S, D] output.

Notes on the math: the reference adds the source mask per-QUERY (constant
along the key axis) before a softmax over keys, so the mask cancels exactly;
encoder_output_embedding and the target mask are unused by the reference.
The kernel therefore computes pure softmax(q k^T / sqrt(dh)) v.

Schedule notes:
- V projection runs d-outer in 4 groups of 4 s-tiles so the first matmul
  only needs 2 input chunks; input DMAs are issued alternately from the
  sync and gpsimd queues (descriptor generation is ~0.7us serial per
  engine, and was the ramp bottleneck).
- The per-qb softmax normalization chain (row-sum -> scatter -> reciprocal
  -> broadcast -> multiply -> store) is drip-fed into the NEXT qb's k-loop
  so its DMA latency hides and the DVE queue never head-of-line blocks.
- Output and the reciprocal broadcast are f16 (half the DMA bytes).
- The scalar engine (exp) is the pacing resource once attention starts;
  heads 1-3 project inside the previous head's exp-paced window.
"""

import math
from contextlib import ExitStack

import numpy as np

import concourse.bacc as bacc
import concourse.tile as tile
from concourse import mybir
from concourse.bass_utils import run_bass_kernel_spmd

N_CORES = 8
B, S, D, H = 4, 2048, 1024, 8
DH = 128                    # head dim
HPC = 4                     # heads per core
DHG = HPC * DH              # 512: projected width per core
SCALE = 1.0 / math.sqrt(DH)
KT = S // 128               # 16 key tiles
ND = D // 128               # 8 contraction tiles
NSB = S // 512              # 4 column blocks of x^T

F32 = mybir.dt.float32
F16 = mybir.dt.float16

TRACE = False               # test.py flips this for profiling runs
_CACHE = {}


def _emit(tc, nc, xt_ap, wq_ap, wk_ap, wv_ap, out_ap):
    with ExitStack() as ctx:
        p_xt = ctx.enter_context(tc.tile_pool(name="xt", bufs=32))
        p_w = ctx.enter_context(tc.tile_pool(name="w", bufs=ND))
        p_qt = ctx.enter_context(tc.tile_pool(name="qt", bufs=2))
        p_v = ctx.enter_context(tc.tile_pool(name="v", bufs=KT))
        p_exp = ctx.enter_context(tc.tile_pool(name="exp", bufs=5))
        p_out = ctx.enter_context(tc.tile_pool(name="o", bufs=2))
        p_rc = ctx.enter_context(tc.tile_pool(name="rc", bufs=2))
        p_const = ctx.enter_context(tc.tile_pool(name="const", bufs=1))
        p_dram = ctx.enter_context(tc.tile_pool(name="dram", bufs=2, space="DRAM"))

        ones = p_const.tile([128, 1], F16, tag="ones")
        nc.vector.memset(ones[:], 1.0)
        # all-ones stationary: replicates the cross-partition sum on all
        # 128 out partitions (free broadcast). Only used for the LAST
        # tile: elsewhere the full-bank sum tiles would block the next
        # window's projection-drip PSUM slots.
        ones_bc = p_const.tile([128, 128], F16, tag="ones_bc")
        nc.vector.memset(ones_bc[:], 1.0)

        # Input DMAs. Descriptor generation is serial per issuing engine
        # (~0.7us each), so alternate sync/gpsimd and order strictly by
        # first use: V group g needs wv[*] + xt[*][g]; q-projection needs
        # wq + all xt; k-projection needs wk.
        xts = [[None] * NSB for _ in range(ND)]
        ws = {"wv": [None] * ND, "wq": [None] * ND, "wk": [None] * ND}

        def dma_w(eng, name, ap, d, split=False):
            t = p_w.tile([128, DHG], F16, tag=name, name=name)
            if split:       # two 64KB halves land ~1.1us sooner than 128KB
                eng.dma_start(t[0:64, :], ap[d * 128:d * 128 + 64, :])
                eng.dma_start(t[64:128, :], ap[d * 128 + 64:(d + 1) * 128, :])
            else:
                eng.dma_start(t[:], ap[d * 128:(d + 1) * 128, :])
            ws[name][d] = t

        def dma_xt(eng, d, sb, split=False):
            t = p_xt.tile([128, 512], F16, tag="xt", name="xt")
            src = xt_ap[d * 128:(d + 1) * 128, sb * 512:(sb + 1) * 512]
            if split:
                eng.dma_start(t[0:64, :], src[0:64, :])
                eng.dma_start(t[64:128, :], src[64:128, :])
            else:
                eng.dma_start(t[:], src)
            xts[d][sb] = t

        for d in range(ND):             # first pair split: it gates MM #1
            dma_w(nc.sync, "wv", wv_ap, d, split=(d == 0))
            dma_xt(nc.gpsimd, d, 0, split=(d == 0))
        for d in range(ND // 2):        # sb1 on both queues: V group 1
            dma_xt(nc.sync, d, 1)       # needs it complete ~19.5us
            dma_xt(nc.gpsimd, d + ND // 2, 1)
        for d in range(ND // 2):
            dma_w(nc.sync, "wq", wq_ap, d)
            dma_w(nc.gpsimd, "wq", wq_ap, d + ND // 2)
        for d in range(ND):
            dma_xt(nc.sync, d, 2)
            dma_xt(nc.gpsimd, d, 3)
        for d in range(ND):
            dma_w(nc.sync, "wk", wk_ap, d)

        # V = x @ wv in natural [s, hd] layout (f16 for the PV matmul).
        # d-outer within groups of 4 s-tiles: the first matmul needs only
        # wv[0] + xt[0][0] (256KB), so the PE starts ~5us earlier than a
        # tile-outer order that needs a full 2MB.
        vts = []
        with tc.tile_pool(name="psv", bufs=8, space="PSUM") as ps_v:
            for g in range(NSB):
                vg = [ps_v.tile([128, DHG], F32, tag="v", name="vps")
                      for _ in range(4)]
                for d in range(ND):
                    for t in range(4):
                        nc.tensor.matmul(
                            vg[t][:],
                            xts[d][g][:, t * 128:(t + 1) * 128],
                            ws["wv"][d][:],
                            start=(d == 0),
                            stop=(d == ND - 1),
                        )
                for t in range(4):
                    vt = p_v.tile([128, DHG], F16, tag="v", name="vt")
                    nc.vector.tensor_copy(vt[:], vg[t][:])
                    vts.append(vt)

        ps_mm = ctx.enter_context(tc.tile_pool(name="psmm", bufs=2, space="PSUM"))
        ps_pv = ctx.enter_context(tc.tile_pool(name="pspv", bufs=1, space="PSUM"))
        ps_pj = ctx.enter_context(tc.tile_pool(name="pspj", bufs=2, space="PSUM"))

        def proj_steps(h):
            """Yield once per PE-chunk of head h's q/k projections."""
            qt = p_qt.tile([128, S], F16, tag="qt", name="qt")
            kt = p_qt.tile([128, S], F16, tag="kt", name="kt")
            for dst, wname, scale in ((qt, "wq", SCALE), (kt, "wk", None)):
                for sb in range(NSB):
                    ps = ps_pj.tile([128, 512], F32, tag="proj", name="pj")
                    for d in range(ND):
                        nc.tensor.matmul(
                            ps[:],
                            ws[wname][d][:, h * 128:(h + 1) * 128],
                            xts[d][sb][:],
                            start=(d == 0),
                            stop=(d == ND - 1),
                        )
                        if d % 2 == 1:
                            yield None
                    dsl = dst[:, sb * 512:(sb + 1) * 512]
                    if scale is not None:
                        nc.vector.tensor_scalar_mul(dsl, ps[:], scale)
                    else:
                        nc.vector.tensor_copy(dsl, ps[:])
            while True:
                yield (qt, kt)

        def tail_steps(h, qb, pv, sms, last):
            """Normalization chain for one (head, qb) tile, emitted lazily.

            Pumped one op per k-step of the NEXT qb so the DMA latency in
            the scatter/broadcast hops hides under compute and the DVE
            queue never head-of-line blocks on them.
            """
            ob = p_out.tile([128, 1024], F16, tag="o", name="ob")
            nc.vector.tensor_copy(ob[:], pv[:])
            yield None
            if last:
                rcb = p_rc.tile([128, 1024], F32, tag="rcb", name="rcb")
                for hf in range(2):
                    nc.vector.reciprocal(rcb[:, hf * 512:(hf + 1) * 512], sms[hf][:])
                nc.vector.tensor_mul(ob[:], ob[:], rcb[:])
                for hf in range(2):
                    nc.sync.dma_start(
                        out_ap[h * 128:(h + 1) * 128,
                               qb * 1024 + hf * 512:qb * 1024 + (hf + 1) * 512],
                        ob[:, hf * 512:(hf + 1) * 512],
                    )
                return
            sm_sb = p_rc.tile([1, 1024], F32, tag="sm_sb", name="sm_sb")
            for hf in range(2):
                nc.vector.tensor_copy(sm_sb[:, hf * 512:(hf + 1) * 512], sms[hf][:])
            yield None
            sm2 = p_rc.tile([128, 8], F32, tag="sm2", name="sm2")
            nc.sync.dma_start(sm2[:], sm_sb[:], single_packet=True)
            yield None
            rc2 = p_rc.tile([128, 8], F32, tag="rc2", name="rc2")
            nc.vector.reciprocal(rc2[:], sm2[:])
            rc2h = p_rc.tile([128, 8], F16, tag="rc2h", name="rc2h")
            nc.vector.tensor_copy(rc2h[:], rc2[:])
            yield None
            r2d = p_dram.tile([1, 1024], F16, tag="r2d", name="r2d")
            nc.sync.dma_start(
                r2d[:].rearrange("a (p c) -> (a p) c", p=128), rc2h[:],
                single_packet=True,
            )
            yield None
            rbc = p_rc.tile([128, 1024], F16, tag="rbc", name="rbc")
            if last:
                for hf in range(2):
                    nc.sync.dma_start(
                        rbc[:, hf * 512:(hf + 1) * 512],
                        r2d[0:1, hf * 512:(hf + 1) * 512].to_broadcast((128, 512)),
                    )
            else:
                nc.sync.dma_start(rbc[:], r2d[0:1, :].to_broadcast((128, 1024)))
            yield None
            nc.vector.tensor_mul(ob[:], ob[:], rbc[:])
            yield None
            if last:
                for hf in range(2):
                    nc.sync.dma_start(
                        out_ap[h * 128:(h + 1) * 128,
                               qb * 1024 + hf * 512:qb * 1024 + (hf + 1) * 512],
                        ob[:, hf * 512:(hf + 1) * 512],
                    )
            else:
                nc.sync.dma_start(
                    out_ap[h * 128:(h + 1) * 128, qb * 1024:(qb + 1) * 1024],
                    ob[:],
                )

        def attention_head(h, qt, kt, next_proj, tail):
            """Phase B for head h; drip-feeds the next head's projection
            matmuls and the previous qb's normalization chain into the
            exp-paced kt loop. Returns this head's last tail generator."""
            for qb in range(S // 1024):
                pv = ps_pv.tile([128, 1024], F32, tag="pv", name="pv")
                q0 = qb * 1024
                ets = {}
                acc = [None]

                def qk_step(k):
                    st_ps = ps_mm.tile([128, 1024], F32, tag="sT", name="sT")
                    for hf in range(2):
                        nc.tensor.matmul(
                            st_ps[:, hf * 512:(hf + 1) * 512],
                            kt[:, k * 128:(k + 1) * 128],
                            qt[:, q0 + hf * 512:q0 + (hf + 1) * 512],
                            start=True,
                            stop=True,
                        )
                    et = p_exp.tile([128, 1024], F16, tag="exp", name="et")
                    nc.scalar.activation(
                        et[:], st_ps[:], mybir.ActivationFunctionType.Exp
                    )
                    ets[k] = et

                def pv_step(k):
                    et = ets.pop(k)
                    for hf in range(2):
                        sl = slice(hf * 512, (hf + 1) * 512)
                        nc.tensor.matmul(
                            pv[:, sl],
                            vts[k][:, h * 128:(h + 1) * 128],
                            et[:, sl],
                            start=(k == 0),
                            stop=(k == KT - 1),
                        )
                    if acc[0] is None:
                        acc[0] = et
                    else:
                        nc.vector.tensor_add(acc[0][:], acc[0][:], et[:])

                qk_step(0)
                qk_step(1)
                for k in range(2, KT):
                    # tail ops 1-6 (copies/DMA issues) early; the multiply
                    # and store wait until s=12 when the rbc broadcast has
                    # landed -- pumped earlier, the multiply blocks the DVE
                    # ~2.6us mid-window, drifting the folds late and
                    # bubbling the NEXT window's first exps behind the
                    # fold-gated row-sum matmuls.
                    s0 = k - 2
                    if tail is not None and (s0 <= 5 or s0 >= 12):
                        if next(tail, StopIteration) is StopIteration:
                            tail = None
                    pv_step(k - 2)
                    qk_step(k)
                    if next_proj is not None:
                        next(next_proj)
                pv_step(KT - 2)
                pv_step(KT - 1)
                while tail is not None:
                    if next(tail, StopIteration) is StopIteration:
                        tail = None

                # cross-partition reduce of the folded exp accumulator
                last = h == HPC - 1 and qb == 1
                sms = []
                for hf in range(2):
                    sl = slice(hf * 512, (hf + 1) * 512)
                    if last:
                        sm = ps_pj.tile([128, 512], F32, tag="proj", name="sm")
                        nc.tensor.matmul(sm[:], ones_bc[:], acc[0][:, sl],
                                         start=True, stop=True)
                    else:
                        sm = ps_pj.tile([1, 512], F32, tag="proj", name="sm")
                        nc.tensor.matmul(sm[:], ones[:], acc[0][:, sl],
                                         start=True, stop=True)
                    sms.append(sm)

                tail = tail_steps(h, qb, pv, sms, last)
            return tail

        # head 0's projections run serially (nothing to hide them under);
        # heads 1..3 project inside the previous head's attention loop.
        gen = proj_steps(0)
        res = None
        while not isinstance(res, tuple):
            res = next(gen)
        qt, kt = res
        tail = None
        for h in range(HPC):
            nxt = proj_steps(h + 1) if h + 1 < HPC else None
            tail = attention_head(h, qt, kt, nxt, tail)
            if nxt is not None:
                res = None
                while not isinstance(res, tuple):
                    res = next(nxt)
                qt, kt = res
        # last tile's normalization chain
        while tail is not None:
            if next(tail, StopIteration) is StopIteration:
                tail = None


def _build():
    nc = bacc.Bacc(
        "TRN2",
        target_bir_lowering=False,
        debug=False,
        enable_asserts=False,
        num_devices=N_CORES,
    )
    xt_ap = nc.dram_tensor("xt", [D, S], F16, kind="ExternalInput").ap()
    wq_ap = nc.dram_tensor("wq", [D, DHG], F16, kind="ExternalInput").ap()
    wk_ap = nc.dram_tensor("wk", [D, DHG], F16, kind="ExternalInput").ap()
    wv_ap = nc.dram_tensor("wv", [D, DHG], F16, kind="ExternalInput").ap()
    out_ap = nc.dram_tensor("out", [DHG, S], F16, kind="ExternalOutput").ap()
    with tile.TileContext(nc) as tc:
        _emit(tc, nc, xt_ap, wq_ap, wk_ap, wv_ap, out_ap)
    nc.compile()
    return nc


def _shard_inputs(inputs):
    x = np.ascontiguousarray(np.asarray(inputs["input_embeddings"], dtype=np.float32))
    wq = np.asarray(inputs["w_query"], dtype=np.float32)
    wk = np.asarray(inputs["w_key"], dtype=np.float32)
    wv = np.asarray(inputs["w_value"], dtype=np.float32)

    def gather(w, g):
        # head h occupies the strided cols d = hd*8 + h; regroup head-major
        w4 = w.reshape(D, DH, H)[:, :, g * HPC:(g + 1) * HPC]   # (D, hd, hl)
        return np.ascontiguousarray(w4.transpose(0, 2, 1).reshape(D, DHG).astype(np.float16))

    in_maps = []
    for c in range(N_CORES):
        b, g = divmod(c, 2)
        in_maps.append(
            {
                "xt": np.ascontiguousarray(x[b].T.astype(np.float16)),
                "wq": gather(wq, g),
                "wk": gather(wk, g),
                "wv": gather(wv, g),
            }
        )
    return in_maps


def kernel(**inputs):
    nc = _CACHE.get("nc")
    if nc is None:
        nc = _CACHE["nc"] = _build()
    in_maps = _shard_inputs(inputs)
    res = run_bass_kernel_spmd(
        nc, in_maps, core_ids=list(range(N_CORES)), trace=TRACE
    )
    _CACHE["last_result"] = res
    out = np.empty((B, S, DH, H), dtype=np.float32)
    for c in range(N_CORES):
        b, g = divmod(c, 2)
        o = res.results[c]["out"].astype(np.float32).reshape(HPC, DH, S)  # (hl, hd, s)
        out[b, :, :, g * HPC:(g + 1) * HPC] = o.transpose(2, 1, 0)
    return out.reshape(B, S, D)


# revision 39
# speedup vs baseline: 1.1822x; 1.1822x over previous
"""Multi-head self-attention (B=4, S=2048, D=1024, H=8) on 8 TRN2 NeuronCores.

Sharding: core c -> batch b=c//2, head-group g=c%2 (4 heads/core).
Each core computes its 4 heads' attention output [512, 2048] (transposed,
head-major); the host gathers/reassembles the full [B, S, D] output.

Notes on the math: the reference adds the source mask per-QUERY (constant
along the key axis) before a softmax over keys, so the mask cancels exactly;
encoder_output_embedding and the target mask are unused by the reference.
The kernel therefore computes pure softmax(q k^T / sqrt(dh)) v.

Schedule notes:
- V projection runs d-outer in 4 groups of 4 s-tiles so the first matmul
  only needs 2 input chunks; input DMAs are issued alternately from the
  sync and gpsimd queues (descriptor generation is ~0.7us serial per
  engine, and was the ramp bottleneck).
- The per-qb softmax normalization chain (row-sum -> scatter -> reciprocal
  -> broadcast -> multiply -> store) is drip-fed into the NEXT qb's k-loop
  so its DMA latency hides and the DVE queue never head-of-line blocks.
- Output and the reciprocal broadcast are f16 (half the DMA bytes).
- The scalar engine (exp) is the pacing resource once attention starts;
  heads 1-3 project inside the previous head's exp-paced window.
"""

import math
from contextlib import ExitStack

import numpy as np

import concourse.bacc as bacc
import concourse.tile as tile
from concourse import mybir
from concourse.bass_utils import run_bass_kernel_spmd

N_CORES = 8
B, S, D, H = 4, 2048, 1024, 8
DH = 128                    # head dim
HPC = 4                     # heads per core
DHG = HPC * DH              # 512: projected width per core
SCALE = 1.0 / math.sqrt(DH)
KT = S // 128               # 16 key tiles
ND = D // 128               # 8 contraction tiles
NSB = S // 512              # 4 column blocks of x^T

F32 = mybir.dt.float32
F16 = mybir.dt.float16

TRACE = False               # test.py flips this for profiling runs
_CACHE = {}


def _emit(tc, nc, xt_ap, wq_ap, wk_ap, wv_ap, out_ap):
    with ExitStack() as ctx:
        p_xt = ctx.enter_context(tc.tile_pool(name="xt", bufs=32))
        p_w = ctx.enter_context(tc.tile_pool(name="w", bufs=ND))
        p_qt = ctx.enter_context(tc.tile_pool(name="qt", bufs=2))
        p_v = ctx.enter_context(tc.tile_pool(name="v", bufs=KT))
        p_exp = ctx.enter_context(tc.tile_pool(name="exp", bufs=5))
        p_out = ctx.enter_context(tc.tile_pool(name="o", bufs=2))
        p_rc = ctx.enter_context(tc.tile_pool(name="rc", bufs=2))
        p_const = ctx.enter_context(tc.tile_pool(name="const", bufs=1))
        p_dram = ctx.enter_context(tc.tile_pool(name="dram", bufs=2, space="DRAM"))

        ones = p_const.tile([128, 1], F16, tag="ones")
        nc.vector.memset(ones[:], 1.0)
        # all-ones stationary: replicates the cross-partition sum on all
        # 128 out partitions (free broadcast). Only used for the LAST
        # tile: elsewhere the full-bank sum tiles would block the next
        # window's projection-drip PSUM slots.
        ones_bc = p_const.tile([128, 128], F16, tag="ones_bc")
        nc.vector.memset(ones_bc[:], 1.0)

        # Input DMAs. Descriptor generation is serial per issuing engine
        # (~0.7us each), so alternate sync/gpsimd and order strictly by
        # first use: V group g needs wv[*] + xt[*][g]; q-projection needs
        # wq + all xt; k-projection needs wk.
        xts = [[None] * NSB for _ in range(ND)]
        ws = {"wv": [None] * ND, "wq": [None] * ND, "wk": [None] * ND}

        def dma_w(eng, name, ap, d, split=False):
            t = p_w.tile([128, DHG], F16, tag=name, name=name)
            if split:       # two 64KB halves land ~1.1us sooner than 128KB
                eng.dma_start(t[0:64, :], ap[d * 128:d * 128 + 64, :])
                eng.dma_start(t[64:128, :], ap[d * 128 + 64:(d + 1) * 128, :])
            else:
                eng.dma_start(t[:], ap[d * 128:(d + 1) * 128, :])
            ws[name][d] = t

        def dma_xt(eng, d, sb, split=False):
            t = p_xt.tile([128, 512], F16, tag="xt", name="xt")
            src = xt_ap[d * 128:(d + 1) * 128, sb * 512:(sb + 1) * 512]
            if split:
                eng.dma_start(t[0:64, :], src[0:64, :])
                eng.dma_start(t[64:128, :], src[64:128, :])
            else:
                eng.dma_start(t[:], src)
            xts[d][sb] = t

        for d in range(ND):             # first pair split: it gates MM #1
            dma_w(nc.sync, "wv", wv_ap, d, split=(d == 0))
            dma_xt(nc.gpsimd, d, 0, split=(d == 0))
        for d in range(ND // 2):        # sb1 on both queues: V group 1
            dma_xt(nc.sync, d, 1)       # needs it complete ~19.5us
            dma_xt(nc.gpsimd, d + ND // 2, 1)
        for d in range(ND // 2):
            dma_w(nc.sync, "wq", wq_ap, d)
            dma_w(nc.gpsimd, "wq", wq_ap, d + ND // 2)
        for d in range(ND):
            dma_xt(nc.sync, d, 2)
            dma_xt(nc.gpsimd, d, 3)
        for d in range(ND):
            dma_w(nc.sync, "wk", wk_ap, d)

        # V = x @ wv in natural [s, hd] layout (f16 for the PV matmul).
        # d-outer within groups of 4 s-tiles: the first matmul needs only
        # wv[0] + xt[0][0] (256KB), so the PE starts ~5us earlier than a
        # tile-outer order that needs a full 2MB.
        vts = []
        with tc.tile_pool(name="psv", bufs=8, space="PSUM") as ps_v:
            for g in range(NSB):
                vg = [ps_v.tile([128, DHG], F32, tag="v", name="vps")
                      for _ in range(4)]
                for d in range(ND):
                    for t in range(4):
                        nc.tensor.matmul(
                            vg[t][:],
                            xts[d][g][:, t * 128:(t + 1) * 128],
                            ws["wv"][d][:],
                            start=(d == 0),
                            stop=(d == ND - 1),
                        )
                for t in range(4):
                    vt = p_v.tile([128, DHG], F16, tag="v", name="vt")
                    nc.vector.tensor_copy(vt[:], vg[t][:])
                    vts.append(vt)

        ps_mm = ctx.enter_context(tc.tile_pool(name="psmm", bufs=2, space="PSUM"))
        ps_pv = ctx.enter_context(tc.tile_pool(name="pspv", bufs=1, space="PSUM"))
        ps_pj = ctx.enter_context(tc.tile_pool(name="pspj", bufs=2, space="PSUM"))

        def proj_steps(h):
            """Yield once per PE-chunk of head h's q/k projections."""
            qt = p_qt.tile([128, S], F16, tag="qt", name="qt")
            kt = p_qt.tile([128, S], F16, tag="kt", name="kt")
            for dst, wname, scale in ((qt, "wq", SCALE), (kt, "wk", None)):
                for sb in range(NSB):
                    ps = ps_pj.tile([128, 512], F32, tag="proj", name="pj")
                    for d in range(ND):
                        nc.tensor.matmul(
                            ps[:],
                            ws[wname][d][:, h * 128:(h + 1) * 128],
                            xts[d][sb][:],
                            start=(d == 0),
                            stop=(d == ND - 1),
                        )
                        if d % 2 == 1:
                            yield None
                    dsl = dst[:, sb * 512:(sb + 1) * 512]
                    if scale is not None:
                        nc.vector.tensor_scalar_mul(dsl, ps[:], scale)
                    else:
                        nc.vector.tensor_copy(dsl, ps[:])
            while True:
                yield (qt, kt)

        def tail_steps(h, qb, pv, sms, last):
            """Normalization chain for one (head, qb) tile, emitted lazily.

            Pumped one op per k-step of the NEXT qb so the DMA latency in
            the scatter/broadcast hops hides under compute and the DVE
            queue never head-of-line blocks on them.
            """
            sm_done = False
            if not last:
                sm_sb = p_rc.tile([1, 1024], F32, tag="sm_sb", name="sm_sb")
                for hf in range(2):
                    nc.vector.tensor_copy(sm_sb[:, hf * 512:(hf + 1) * 512],
                                          sms[hf][:])
                sm_done = True
            ob = p_out.tile([128, 1024], F16, tag="o", name="ob")
            nc.vector.tensor_copy(ob[:], pv[:])
            yield None
            if last:
                rcb = p_rc.tile([128, 1024], F32, tag="rcb", name="rcb")
                for hf in range(2):
                    nc.vector.reciprocal(rcb[:, hf * 512:(hf + 1) * 512], sms[hf][:])
                nc.vector.tensor_mul(ob[:], ob[:], rcb[:])
                for hf in range(2):
                    nc.sync.dma_start(
                        out_ap[h * 128:(h + 1) * 128,
                               qb * 1024 + hf * 512:qb * 1024 + (hf + 1) * 512],
                        ob[:, hf * 512:(hf + 1) * 512],
                    )
                return
            yield None
            sm2 = p_rc.tile([128, 8], F32, tag="sm2", name="sm2")
            nc.sync.dma_start(sm2[:], sm_sb[:], single_packet=True)
            yield None
            rc2 = p_rc.tile([128, 8], F32, tag="rc2", name="rc2")
            nc.vector.reciprocal(rc2[:], sm2[:])
            rc2h = p_rc.tile([128, 8], F16, tag="rc2h", name="rc2h")
            nc.vector.tensor_copy(rc2h[:], rc2[:])
            yield None
            r2d = p_dram.tile([1, 1024], F16, tag="r2d", name="r2d")
            nc.sync.dma_start(
                r2d[:].rearrange("a (p c) -> (a p) c", p=128), rc2h[:],
                single_packet=True,
            )
            yield None
            rbc = p_rc.tile([128, 1024], F16, tag="rbc", name="rbc")
            if last:
                for hf in range(2):
                    nc.sync.dma_start(
                        rbc[:, hf * 512:(hf + 1) * 512],
                        r2d[0:1, hf * 512:(hf + 1) * 512].to_broadcast((128, 512)),
                    )
            else:
                nc.sync.dma_start(rbc[:], r2d[0:1, :].to_broadcast((128, 1024)))
            yield None
            nc.vector.tensor_mul(ob[:], ob[:], rbc[:])
            yield None
            if last:
                for hf in range(2):
                    nc.sync.dma_start(
                        out_ap[h * 128:(h + 1) * 128,
                               qb * 1024 + hf * 512:qb * 1024 + (hf + 1) * 512],
                        ob[:, hf * 512:(hf + 1) * 512],
                    )
            else:
                nc.sync.dma_start(
                    out_ap[h * 128:(h + 1) * 128, qb * 1024:(qb + 1) * 1024],
                    ob[:],
                )

        def attention_head(h, qt, kt, next_proj, tail):
            """Phase B for head h; drip-feeds the next head's projection
            matmuls and the previous qb's normalization chain into the
            exp-paced kt loop. Returns this head's last tail generator."""
            for qb in range(S // 1024):
                pv = ps_pv.tile([128, 1024], F32, tag="pv", name="pv")
                q0 = qb * 1024
                ets = {}
                acc = [None]

                def qk_step(k):
                    st_ps = ps_mm.tile([128, 1024], F32, tag="sT", name="sT")
                    for hf in range(2):
                        nc.tensor.matmul(
                            st_ps[:, hf * 512:(hf + 1) * 512],
                            kt[:, k * 128:(k + 1) * 128],
                            qt[:, q0 + hf * 512:q0 + (hf + 1) * 512],
                            start=True,
                            stop=True,
                        )
                    et = p_exp.tile([128, 1024], F16, tag="exp", name="et")
                    nc.scalar.activation(
                        et[:], st_ps[:], mybir.ActivationFunctionType.Exp
                    )
                    ets[k] = et

                def pv_step(k):
                    et = ets.pop(k)
                    for hf in range(2):
                        sl = slice(hf * 512, (hf + 1) * 512)
                        nc.tensor.matmul(
                            pv[:, sl],
                            vts[k][:, h * 128:(h + 1) * 128],
                            et[:, sl],
                            start=(k == 0),
                            stop=(k == KT - 1),
                        )
                    if acc[0] is None:
                        acc[0] = et
                    else:
                        nc.vector.tensor_add(acc[0][:], acc[0][:], et[:])

                qk_step(0)
                qk_step(1)
                for k in range(2, KT):
                    # tail ops 1-6 (copies/DMA issues) early; the multiply
                    # and store wait until s=12 when the rbc broadcast has
                    # landed -- pumped earlier, the multiply blocks the DVE
                    # ~2.6us mid-window, drifting the folds late and
                    # bubbling the NEXT window's first exps behind the
                    # fold-gated row-sum matmuls.
                    s0 = k - 2
                    if tail is not None and (s0 <= 5 or s0 >= 12):
                        if next(tail, StopIteration) is StopIteration:
                            tail = None
                    pv_step(k - 2)
                    qk_step(k)
                    if next_proj is not None:
                        next(next_proj)
                pv_step(KT - 2)
                pv_step(KT - 1)
                while tail is not None:
                    if next(tail, StopIteration) is StopIteration:
                        tail = None

                # cross-partition reduce of the folded exp accumulator
                last = h == HPC - 1 and qb == 1
                sms = []
                for hf in range(2):
                    sl = slice(hf * 512, (hf + 1) * 512)
                    if last:
                        sm = ps_pj.tile([128, 512], F32, tag="proj", name="sm")
                        nc.tensor.matmul(sm[:], ones_bc[:], acc[0][:, sl],
                                         start=True, stop=True)
                    else:
                        sm = ps_pj.tile([1, 512], F32, tag="proj", name="sm")
                        nc.tensor.matmul(sm[:], ones[:], acc[0][:, sl],
                                         start=True, stop=True)
                    sms.append(sm)

                tail = tail_steps(h, qb, pv, sms, last)
            return tail

        # head 0's projections run serially (nothing to hide them under);
        # heads 1..3 project inside the previous head's attention loop.
        gen = proj_steps(0)
        res = None
        while not isinstance(res, tuple):
            res = next(gen)
        qt, kt = res
        tail = None
        for h in range(HPC):
            nxt = proj_steps(h + 1) if h + 1 < HPC else None
            tail = attention_head(h, qt, kt, nxt, tail)
            if nxt is not None:
                res = None
                while not isinstance(res, tuple):
                    res = next(nxt)
                qt, kt = res
        # last tile's normalization chain
        while tail is not None:
            if next(tail, StopIteration) is StopIteration:
                tail = None


def _build():
    nc = bacc.Bacc(
        "TRN2",
        target_bir_lowering=False,
        debug=False,
        enable_asserts=False,
        num_devices=N_CORES,
    )
    xt_ap = nc.dram_tensor("xt", [D, S], F16, kind="ExternalInput").ap()
    wq_ap = nc.dram_tensor("wq", [D, DHG], F16, kind="ExternalInput").ap()
    wk_ap = nc.dram_tensor("wk", [D, DHG], F16, kind="ExternalInput").ap()
    wv_ap = nc.dram_tensor("wv", [D, DHG], F16, kind="ExternalInput").ap()
    out_ap = nc.dram_tensor("out", [DHG, S], F16, kind="ExternalOutput").ap()
    with tile.TileContext(nc) as tc:
        _emit(tc, nc, xt_ap, wq_ap, wk_ap, wv_ap, out_ap)
    nc.compile()
    return nc


def _shard_inputs(inputs):
    x = np.ascontiguousarray(np.asarray(inputs["input_embeddings"], dtype=np.float32))
    wq = np.asarray(inputs["w_query"], dtype=np.float32)
    wk = np.asarray(inputs["w_key"], dtype=np.float32)
    wv = np.asarray(inputs["w_value"], dtype=np.float32)

    def gather(w, g):
        # head h occupies the strided cols d = hd*8 + h; regroup head-major
        w4 = w.reshape(D, DH, H)[:, :, g * HPC:(g + 1) * HPC]   # (D, hd, hl)
        return np.ascontiguousarray(w4.transpose(0, 2, 1).reshape(D, DHG).astype(np.float16))

    in_maps = []
    for c in range(N_CORES):
        b, g = divmod(c, 2)
        in_maps.append(
            {
                "xt": np.ascontiguousarray(x[b].T.astype(np.float16)),
                "wq": gather(wq, g),
                "wk": gather(wk, g),
                "wv": gather(wv, g),
            }
        )
    return in_maps


def kernel(**inputs):
    nc = _CACHE.get("nc")
    if nc is None:
        nc = _CACHE["nc"] = _build()
    in_maps = _shard_inputs(inputs)
    res = run_bass_kernel_spmd(
        nc, in_maps, core_ids=list(range(N_CORES)), trace=TRACE
    )
    _CACHE["last_result"] = res
    out = np.empty((B, S, DH, H), dtype=np.float32)
    for c in range(N_CORES):
        b, g = divmod(c, 2)
        o = res.results[c]["out"].astype(np.float32).reshape(HPC, DH, S)  # (hl, hd, s)
        out[b, :, :, g * HPC:(g + 1) * HPC] = o.transpose(2, 1, 0)
    return out.reshape(B, S, D)


# revision 40
# speedup vs baseline: 1.1892x; 1.0059x over previous
"""Multi-head self-attention (B=4, S=2048, D=1024, H=8) on 8 TRN2 NeuronCores.

Sharding: core c -> batch b=c//2, head-group g=c%2 (4 heads/core).
Each core computes its 4 heads' attention output [512, 2048] (transposed,
head-major); the host gathers/reassembles the full [B, S, D] output.

Notes on the math: the reference adds the source mask per-QUERY (constant
along the key axis) before a softmax over keys, so the mask cancels exactly;
encoder_output_embedding and the target mask are unused by the reference.
The kernel therefore computes pure softmax(q k^T / sqrt(dh)) v.

Schedule notes:
- V projection runs d-outer in 4 groups of 4 s-tiles so the first matmul
  only needs 2 input chunks; input DMAs are issued alternately from the
  sync and gpsimd queues (descriptor generation is ~0.7us serial per
  engine, and was the ramp bottleneck).
- The per-qb softmax normalization chain (row-sum -> scatter -> reciprocal
  -> broadcast -> multiply -> store) is drip-fed into the NEXT qb's k-loop
  so its DMA latency hides and the DVE queue never head-of-line blocks.
- Output and the reciprocal broadcast are f16 (half the DMA bytes).
- The scalar engine (exp) is the pacing resource once attention starts;
  heads 1-3 project inside the previous head's exp-paced window.
"""

import math
from contextlib import ExitStack

import numpy as np

import concourse.bacc as bacc
import concourse.tile as tile
from concourse import mybir
from concourse.bass_utils import run_bass_kernel_spmd

N_CORES = 8
B, S, D, H = 4, 2048, 1024, 8
DH = 128                    # head dim
HPC = 4                     # heads per core
DHG = HPC * DH              # 512: projected width per core
SCALE = 1.0 / math.sqrt(DH)
KT = S // 128               # 16 key tiles
ND = D // 128               # 8 contraction tiles
NSB = S // 512              # 4 column blocks of x^T

F32 = mybir.dt.float32
F16 = mybir.dt.float16

TRACE = False               # test.py flips this for profiling runs
_CACHE = {}


def _emit(tc, nc, xt_ap, wq_ap, wk_ap, wv_ap, out_ap):
    with ExitStack() as ctx:
        p_xt = ctx.enter_context(tc.tile_pool(name="xt", bufs=32))
        p_w = ctx.enter_context(tc.tile_pool(name="w", bufs=ND))
        p_qt = ctx.enter_context(tc.tile_pool(name="qt", bufs=2))
        p_v = ctx.enter_context(tc.tile_pool(name="v", bufs=KT))
        p_exp = ctx.enter_context(tc.tile_pool(name="exp", bufs=5))
        p_out = ctx.enter_context(tc.tile_pool(name="o", bufs=2))
        p_rc = ctx.enter_context(tc.tile_pool(name="rc", bufs=2))
        p_const = ctx.enter_context(tc.tile_pool(name="const", bufs=1))
        p_dram = ctx.enter_context(tc.tile_pool(name="dram", bufs=2, space="DRAM"))

        ones = p_const.tile([128, 1], F16, tag="ones")
        nc.vector.memset(ones[:], 1.0)
        # all-ones stationary: replicates the cross-partition sum on all
        # 128 out partitions (free broadcast). Only used for the LAST
        # tile: elsewhere the full-bank sum tiles would block the next
        # window's projection-drip PSUM slots.
        ones_bc = p_const.tile([128, 128], F16, tag="ones_bc")
        nc.vector.memset(ones_bc[:], 1.0)

        # Input DMAs. Descriptor generation is serial per issuing engine
        # (~0.7us each), so alternate sync/gpsimd and order strictly by
        # first use: V group g needs wv[*] + xt[*][g]; q-projection needs
        # wq + all xt; k-projection needs wk.
        xts = [[None] * NSB for _ in range(ND)]
        ws = {"wv": [None] * ND, "wq": [None] * ND, "wk": [None] * ND}

        def dma_w(eng, name, ap, d, split=False):
            t = p_w.tile([128, DHG], F16, tag=name, name=name)
            if split:       # two 64KB halves land ~1.1us sooner than 128KB
                eng.dma_start(t[0:64, :], ap[d * 128:d * 128 + 64, :])
                eng.dma_start(t[64:128, :], ap[d * 128 + 64:(d + 1) * 128, :])
            else:
                eng.dma_start(t[:], ap[d * 128:(d + 1) * 128, :])
            ws[name][d] = t

        def dma_xt(eng, d, sb, split=False):
            t = p_xt.tile([128, 512], F16, tag="xt", name="xt")
            src = xt_ap[d * 128:(d + 1) * 128, sb * 512:(sb + 1) * 512]
            if split:
                eng.dma_start(t[0:64, :], src[0:64, :])
                eng.dma_start(t[64:128, :], src[64:128, :])
            else:
                eng.dma_start(t[:], src)
            xts[d][sb] = t

        for d in range(ND):             # first pair split: it gates MM #1
            dma_w(nc.sync, "wv", wv_ap, d, split=(d == 0))
            dma_xt(nc.gpsimd, d, 0, split=(d == 0))
        for d in range(ND // 2):        # sb1 on both queues: V group 1
            dma_xt(nc.sync, d, 1)       # needs it complete ~19.5us
            dma_xt(nc.gpsimd, d + ND // 2, 1)
        for d in range(ND // 2):
            dma_w(nc.sync, "wq", wq_ap, d)
            dma_w(nc.gpsimd, "wq", wq_ap, d + ND // 2)
        for d in range(ND):
            dma_xt(nc.sync, d, 2)
            dma_xt(nc.gpsimd, d, 3)
        for d in range(ND):
            dma_w(nc.sync, "wk", wk_ap, d)

        # V = x @ wv in natural [s, hd] layout (f16 for the PV matmul).
        # d-outer within groups of 4 s-tiles: the first matmul needs only
        # wv[0] + xt[0][0] (256KB), so the PE starts ~5us earlier than a
        # tile-outer order that needs a full 2MB.
        vts = []
        with tc.tile_pool(name="psv", bufs=8, space="PSUM") as ps_v:
            for g in range(NSB):
                vg = [ps_v.tile([128, DHG], F32, tag="v", name="vps")
                      for _ in range(4)]
                for d in range(ND):
                    for t in range(4):
                        nc.tensor.matmul(
                            vg[t][:],
                            xts[d][g][:, t * 128:(t + 1) * 128],
                            ws["wv"][d][:],
                            start=(d == 0),
                            stop=(d == ND - 1),
                        )
                for t in range(4):
                    vt = p_v.tile([128, DHG], F16, tag="v", name="vt")
                    nc.vector.tensor_copy(vt[:], vg[t][:])
                    vts.append(vt)

        ps_mm = ctx.enter_context(tc.tile_pool(name="psmm", bufs=2, space="PSUM"))
        ps_pv = ctx.enter_context(tc.tile_pool(name="pspv", bufs=1, space="PSUM"))
        ps_pj = ctx.enter_context(tc.tile_pool(name="pspj", bufs=2, space="PSUM"))

        def proj_steps(h):
            """Yield once per PE-chunk of head h's q/k projections."""
            qt = p_qt.tile([128, S], F16, tag="qt", name="qt")
            kt = p_qt.tile([128, S], F16, tag="kt", name="kt")
            for dst, wname, scale in ((qt, "wq", SCALE), (kt, "wk", None)):
                for sb in range(NSB):
                    ps = ps_pj.tile([128, 512], F32, tag="proj", name="pj")
                    for d in range(ND):
                        nc.tensor.matmul(
                            ps[:],
                            ws[wname][d][:, h * 128:(h + 1) * 128],
                            xts[d][sb][:],
                            start=(d == 0),
                            stop=(d == ND - 1),
                        )
                        if d % 2 == 1:
                            yield None
                    dsl = dst[:, sb * 512:(sb + 1) * 512]
                    if scale is not None:
                        nc.vector.tensor_scalar_mul(dsl, ps[:], scale)
                    else:
                        nc.vector.tensor_copy(dsl, ps[:])
            while True:
                yield (qt, kt)

        def tail_steps(h, qb, pv, sms, last):
            """Normalization chain for one (head, qb) tile, emitted lazily.

            Pumped one op per k-step of the NEXT qb so the DMA latency in
            the scatter/broadcast hops hides under compute and the DVE
            queue never head-of-line blocks on them.
            """
            ob = p_out.tile([128, 1024], F16, tag="o", name="ob")
            nc.vector.tensor_copy(ob[:], pv[:])
            yield None
            if last:
                rcb = p_rc.tile([128, 1024], F32, tag="rcb", name="rcb")
                for hf in range(2):
                    nc.vector.reciprocal(rcb[:, hf * 512:(hf + 1) * 512], sms[hf][:])
                nc.vector.tensor_mul(ob[:], ob[:], rcb[:])
                for hf in range(2):
                    nc.sync.dma_start(
                        out_ap[h * 128:(h + 1) * 128,
                               qb * 1024 + hf * 512:qb * 1024 + (hf + 1) * 512],
                        ob[:, hf * 512:(hf + 1) * 512],
                    )
                return
            sm_sb = p_rc.tile([1, 1024], F32, tag="sm_sb", name="sm_sb")
            for hf in range(2):
                nc.vector.tensor_copy(sm_sb[:, hf * 512:(hf + 1) * 512], sms[hf][:])
            yield None
            sm2 = p_rc.tile([128, 8], F32, tag="sm2", name="sm2")
            nc.sync.dma_start(sm2[:], sm_sb[:], single_packet=True)
            yield None
            rc2 = p_rc.tile([128, 8], F32, tag="rc2", name="rc2")
            nc.vector.reciprocal(rc2[:], sm2[:])
            rc2h = p_rc.tile([128, 8], F16, tag="rc2h", name="rc2h")
            nc.vector.tensor_copy(rc2h[:], rc2[:])
            yield None
            r2d = p_dram.tile([1, 1024], F16, tag="r2d", name="r2d")
            nc.sync.dma_start(
                r2d[:].rearrange("a (p c) -> (a p) c", p=128), rc2h[:],
                single_packet=True,
            )
            yield None
            rbc = p_rc.tile([128, 1024], F16, tag="rbc", name="rbc")
            if last:
                for hf in range(2):
                    nc.sync.dma_start(
                        rbc[:, hf * 512:(hf + 1) * 512],
                        r2d[0:1, hf * 512:(hf + 1) * 512].to_broadcast((128, 512)),
                    )
            else:
                nc.sync.dma_start(rbc[:], r2d[0:1, :].to_broadcast((128, 1024)))
            yield None
            nc.vector.tensor_mul(ob[:], ob[:], rbc[:])
            yield None
            if last:
                for hf in range(2):
                    nc.sync.dma_start(
                        out_ap[h * 128:(h + 1) * 128,
                               qb * 1024 + hf * 512:qb * 1024 + (hf + 1) * 512],
                        ob[:, hf * 512:(hf + 1) * 512],
                    )
            else:
                nc.sync.dma_start(
                    out_ap[h * 128:(h + 1) * 128, qb * 1024:(qb + 1) * 1024],
                    ob[:],
                )

        def attention_head(h, qt, kt, next_proj, tail):
            """Phase B for head h; drip-feeds the next head's projection
            matmuls and the previous qb's normalization chain into the
            exp-paced kt loop. Returns this head's last tail generator."""
            for qb in range(S // 1024):
                pv = ps_pv.tile([128, 1024], F32, tag="pv", name="pv")
                q0 = qb * 1024
                ets = {}
                acc = [None]

                def qk_step(k):
                    st_ps = ps_mm.tile([128, 1024], F32, tag="sT", name="sT")
                    for hf in range(2):
                        nc.tensor.matmul(
                            st_ps[:, hf * 512:(hf + 1) * 512],
                            kt[:, k * 128:(k + 1) * 128],
                            qt[:, q0 + hf * 512:q0 + (hf + 1) * 512],
                            start=True,
                            stop=True,
                        )
                    et = p_exp.tile([128, 1024], F16, tag="exp", name="et")
                    nc.scalar.activation(
                        et[:], st_ps[:], mybir.ActivationFunctionType.Exp
                    )
                    ets[k] = et

                def pv_step(k):
                    et = ets.pop(k)
                    for hf in range(2):
                        sl = slice(hf * 512, (hf + 1) * 512)
                        nc.tensor.matmul(
                            pv[:, sl],
                            vts[k][:, h * 128:(h + 1) * 128],
                            et[:, sl],
                            start=(k == 0),
                            stop=(k == KT - 1),
                        )
                    if acc[0] is None:
                        acc[0] = et
                    else:
                        nc.vector.tensor_add(acc[0][:], acc[0][:], et[:])

                qk_step(0)
                qk_step(1)
                for k in range(2, KT):
                    if tail is not None:
                        if next(tail, StopIteration) is StopIteration:
                            tail = None
                    pv_step(k - 2)
                    qk_step(k)
                    if next_proj is not None:
                        next(next_proj)
                pv_step(KT - 2)
                pv_step(KT - 1)
                while tail is not None:
                    if next(tail, StopIteration) is StopIteration:
                        tail = None

                # cross-partition reduce of the folded exp accumulator
                last = h == HPC - 1 and qb == 1
                sms = []
                for hf in range(2):
                    sl = slice(hf * 512, (hf + 1) * 512)
                    if last:
                        sm = ps_pj.tile([128, 512], F32, tag="proj", name="sm")
                        nc.tensor.matmul(sm[:], ones_bc[:], acc[0][:, sl],
                                         start=True, stop=True)
                    else:
                        sm = ps_pj.tile([1, 512], F32, tag="proj", name="sm")
                        nc.tensor.matmul(sm[:], ones[:], acc[0][:, sl],
                                         start=True, stop=True)
                    sms.append(sm)

                tail = tail_steps(h, qb, pv, sms, last)
            return tail

        # head 0's projections run serially (nothing to hide them under);
        # heads 1..3 project inside the previous head's attention loop.
        gen = proj_steps(0)
        res = None
        while not isinstance(res, tuple):
            res = next(gen)
        qt, kt = res
        tail = None
        for h in range(HPC):
            nxt = proj_steps(h + 1) if h + 1 < HPC else None
            tail = attention_head(h, qt, kt, nxt, tail)
            if nxt is not None:
                res = None
                while not isinstance(res, tuple):
                    res = next(nxt)
                qt, kt = res
        # last tile's normalization chain
        while tail is not None:
            if next(tail, StopIteration) is StopIteration:
                tail = None


def _build():
    nc = bacc.Bacc(
        "TRN2",
        target_bir_lowering=False,
        debug=False,
        enable_asserts=False,
        num_devices=N_CORES,
    )
    xt_ap = nc.dram_tensor("xt", [D, S], F16, kind="ExternalInput").ap()
    wq_ap = nc.dram_tensor("wq", [D, DHG], F16, kind="ExternalInput").ap()
    wk_ap = nc.dram_tensor("wk", [D, DHG], F16, kind="ExternalInput").ap()
    wv_ap = nc.dram_tensor("wv", [D, DHG], F16, kind="ExternalInput").ap()
    out_ap = nc.dram_tensor("out", [DHG, S], F16, kind="ExternalOutput").ap()
    with tile.TileContext(nc) as tc:
        _emit(tc, nc, xt_ap, wq_ap, wk_ap, wv_ap, out_ap)
    nc.compile()
    return nc


def _shard_inputs(inputs):
    x = np.ascontiguousarray(np.asarray(inputs["input_embeddings"], dtype=np.float32))
    wq = np.asarray(inputs["w_query"], dtype=np.float32)
    wk = np.asarray(inputs["w_key"], dtype=np.float32)
    wv = np.asarray(inputs["w_value"], dtype=np.float32)

    def gather(w, g):
        # head h occupies the strided cols d = hd*8 + h; regroup head-major
        w4 = w.reshape(D, DH, H)[:, :, g * HPC:(g + 1) * HPC]   # (D, hd, hl)
        return np.ascontiguousarray(w4.transpose(0, 2, 1).reshape(D, DHG).astype(np.float16))

    in_maps = []
    for c in range(N_CORES):
        b, g = divmod(c, 2)
        in_maps.append(
            {
                "xt": np.ascontiguousarray(x[b].T.astype(np.float16)),
                "wq": gather(wq, g),
                "wk": gather(wk, g),
                "wv": gather(wv, g),
            }
        )
    return in_maps


def kernel(**inputs):
    nc = _CACHE.get("nc")
    if nc is None:
        nc = _CACHE["nc"] = _build()
    in_maps = _shard_inputs(inputs)
    res = run_bass_kernel_spmd(
        nc, in_maps, core_ids=list(range(N_CORES)), trace=TRACE
    )
    _CACHE["last_result"] = res
    out = np.empty((B, S, DH, H), dtype=np.float32)
    for c in range(N_CORES):
        b, g = divmod(c, 2)
        o = res.results[c]["out"].astype(np.float32).reshape(HPC, DH, S)  # (hl, hd, s)
        out[b, :, :, g * HPC:(g + 1) * HPC] = o.transpose(2, 1, 0)
    return out.reshape(B, S, D)
